# revision 1
# baseline (speedup 1.0000x reference)
"""MPNCOV (iSQRT-COV pooling) Trainium2 kernel.

Math per sample (C=256 channels, M=196 spatial):
  xc   = x - mean_m(x)                      # center along spatial dim
  A    = xc @ xc^T / sum(xc^2)              # = cov / trace(cov)
  Newton-Schulz (ITER_N=3) on A, final y = sqrt(normA) * YZY, triu-packed.

Scale folding: every intermediate X is stored as X_s with X = sigma_X * X_s,
sigma tracked symbolically so each PSUM->SBUF transform is a single
tensor_tensor subtract against a constant diagonal tile:
  ZY1_s = 3I   - A_s          (sigma 1/2)
  Y1_s  = A_s @ ZY1_s         (sigma 1/2)
  W1_s  = ZY1_s @ Y1_s        (sigma 1/4)
  ZY2_s = 12I  - W1_s         (sigma 1/8)
  Y2_s  = Y1_s @ ZY2_s        (sigma 1/16)
  Z2_s  = ZY2_s @ ZY1_s       (sigma 1/16)
  W2_s  = Z2_s @ Y2_s         (sigma 1/256)
  ZY3_s = 768I - W2_s
  F_s   = Y2_s @ ZY3_s,   y = (sqrt(tr/M)/8192) * F_s
All intermediates are polynomials in symmetric A => symmetric, so row-tiles
serve directly as matmul lhsT (no transposes in the NS chain). The only PE
transposes build xc^T for the Gram matmul; 1/sqrt(sum xc^2) is folded into
the transpose's PSUM->SBUF copy so the Gram directly yields A_s.

Matrices are stored as single [128, 512] tiles: cols 0:256 = matrix rows
0:128, cols 256:512 = matrix rows 128:256. Each product lands in ONE fp32
PSUM bank [128, 512] (two N=256 matmul groups), so every PSUM->SBUF
transform is one 512-wide DVE/ACT op. Matmul inputs are fp16 (1 cyc/row on
the PE + fast weight load); PSUM accumulation stays fp32.

Sharding: pure data parallel, batch 256 -> 32 samples on each of 8 cores.
Triu packing: all 32 per-sample results stay SBUF-resident; at the end one
DMA per matrix row r moves that row's triu tail for all 32 samples
(constant strides in both src and dst), alternating sync/scalar HWDGE.
"""

import numpy as np

from concourse import bacc, bass, bass_isa, mybir, tile
from concourse import bass_utils

F32 = mybir.dt.float32
P = 128
C = 256
M = 196
B = 256
NCORES = 8
S = B // NCORES            # samples per core
NTRIU = C * (C + 1) // 2   # 32896

# matmul input dtype for the big products
MM_DT = mybir.dt.float16

LAST_EXEC_NS = None
LAST_RESULTS = None


def build(tc, y_ap, x_ap, ident_ap, icons_ap, ones_ap, onesrow_ap, n_samples=S):
    nc = tc.nc
    import contextlib

    with contextlib.ExitStack() as ctx:
        consts = ctx.enter_context(tc.tile_pool(name="consts", bufs=1))
        fpool = ctx.enter_context(tc.tile_pool(name="fpool", bufs=1))
        work = ctx.enter_context(tc.tile_pool(name="work", bufs=3))
        mats = ctx.enter_context(tc.tile_pool(name="mats", bufs=3))
        psum = ctx.enter_context(tc.tile_pool(name="psum", bufs=8, space="PSUM"))

        ident = consts.tile([P, P], MM_DT, tag="ident")
        nc.sync.dma_start(ident[:], ident_ap[:])
        icons = consts.tile([P, 3, 2 * C], MM_DT, tag="icons")
        nc.sync.dma_start(icons[:], icons_ap[:])
        ones = consts.tile([P, 1], F32, tag="ones")
        nc.sync.dma_start(ones[:], ones_ap[:])
        onesrow = consts.tile([1, P], F32, tag="onesrow")
        nc.sync.dma_start(onesrow[:], onesrow_ap[:])

        ftiles = [
            fpool.tile([P, n_samples, C], F32, tag=f"F_m{mt}", name=f"F_m{mt}")
            for mt in range(2)
        ]

        rowstart = np.concatenate([[0], np.cumsum(C - np.arange(C))]).astype(np.int64)

        def prod(U, V):
            """One [128,512] PSUM bank <- U @ V (both [P,512] fp16, symmetric)."""
            p_t = psum.tile([P, 2 * C], F32, tag="ps_big")
            for mt in range(2):
                oc = slice(mt * C, (mt + 1) * C)
                ms0 = slice(mt * P, mt * P + P)
                ms1 = slice(C + mt * P, C + mt * P + P)
                nc.tensor.matmul(
                    p_t[:, oc], U[:, ms0], V[:, 0:C], start=True, stop=False
                )
                nc.tensor.matmul(
                    p_t[:, oc], U[:, ms1], V[:, C : 2 * C], start=False, stop=True
                )
            return p_t

        def sample_stages(b):
            """Yield closures for one sample's pipeline stages; tiles tagged
            by b%2 so a pair of samples uses disjoint pool slots and their
            PE bursts interleave (keeps the PE dense enough to stay warm)."""
            x = {}
            fx = f"_{b % 3}"

            def load():
                x["xr"] = work.tile([P, 2, M], F32, tag="xr" + fx, name="xr" + fx)
                nc.sync.dma_start(
                    x["xr"][:], x_ap[b].rearrange("(h p) m -> p h m", p=P)
                )

            def stats():
                xr = x["xr"]
                mean2 = work.tile([P, 2], F32, tag="mean2" + fx, name="mean2" + fx)
                nc.vector.tensor_reduce(
                    mean2[:], xr[:], axis=mybir.AxisListType.X,
                    op=mybir.AluOpType.add,
                )
                negmean = work.tile([P, 2], F32, tag="negmean" + fx, name="nm" + fx)
                nc.vector.tensor_scalar_mul(negmean[:], mean2[:], -1.0 / M)
                xc = work.tile([P, 2, M], MM_DT, tag="xc" + fx, name="xc" + fx)
                sq = work.tile([P, 2, M], MM_DT, tag="sq" + fx, name="sq" + fx)
                s2 = work.tile([P, 2], F32, tag="s2" + fx, name="s2" + fx)
                for h in range(2):
                    nc.vector.tensor_scalar_add(
                        xc[:, h], xr[:, h], negmean[:, h : h + 1]
                    )
                    nc.scalar.activation(
                        sq[:, h], xc[:, h],
                        mybir.ActivationFunctionType.Square,
                        accum_out=s2[:, h : h + 1],
                    )
                x["xc"], x["s2"] = xc, s2

            def trace():
                s2 = x["s2"]
                s2r = work.tile([P, 2], F32, tag="s2r" + fx, name="s2r" + fx)
                nc.gpsimd.partition_all_reduce(
                    s2r[:], s2[:], channels=P, reduce_op=bass_isa.ReduceOp.add
                )
                trv = work.tile([P, 1], F32, tag="trv" + fx, name="trv" + fx)
                nc.vector.tensor_tensor(
                    trv[:], s2r[:, 0:1], s2r[:, 1:2], op=mybir.AluOpType.add
                )
                abv = work.tile([P, 2], F32, tag="abv" + fx, name="abv" + fx)
                inv = work.tile([P, 1], F32, tag="inv" + fx, name="inv" + fx)
                nc.vector.reciprocal(inv[:], trv[:])
                nc.scalar.activation(
                    abv[:, 0:1], inv[:], mybir.ActivationFunctionType.Sqrt,
                    scale=1.0,
                )
                nc.scalar.activation(
                    abv[:, 1:2], trv[:], mybir.ActivationFunctionType.Sqrt,
                    scale=1.0 / (M * 8192.0 * 8192.0),
                )
                x["abv"] = abv

            def transpose():
                xc = x["xc"]
                tp = psum.tile([P, 2 * C], MM_DT, tag="ps_big", name="tp" + fx)
                for h in range(2):
                    nc.tensor.transpose(
                        tp[:, h * P : h * P + P], xc[:, h, 0:P], ident[:]
                    )
                    nc.tensor.transpose(
                        tp[0 : M - P, C + h * P : C + h * P + P], xc[:, h, P:M],
                        ident[:],
                    )
                x["tp"] = tp

            def scale_xcT():
                tp, abv = x["tp"], x["abv"]
                xcT0 = work.tile([P, C], MM_DT, tag="xcT0" + fx, name="xcT0" + fx)
                xcT1 = work.tile([P, C], MM_DT, tag="xcT1" + fx, name="xcT1" + fx)
                nc.vector.tensor_scalar_mul(xcT0[:], tp[:, 0:C], abv[:, 0:1])
                nc.vector.tensor_scalar_mul(
                    xcT1[0 : M - P], tp[0 : M - P, C : 2 * C],
                    abv[0 : M - P, 0:1],
                )
                x["xcT0"], x["xcT1"] = xcT0, xcT1

            def gram():
                xcT0, xcT1 = x["xcT0"], x["xcT1"]
                a_ps = psum.tile([P, 2 * C], F32, tag="ps_big", name="aps" + fx)
                for mt in range(2):
                    oc = slice(mt * C, (mt + 1) * C)
                    ms = slice(mt * P, (mt + 1) * P)
                    nc.tensor.matmul(
                        a_ps[:, oc], xcT0[:, ms], xcT0[:], start=True, stop=False
                    )
                    nc.tensor.matmul(
                        a_ps[:, oc], xcT1[0 : M - P, ms], xcT1[0 : M - P, :],
                        start=False, stop=True,
                    )
                x["a_ps"] = a_ps

            def mat(tag):
                t = mats.tile([P, 2 * C], MM_DT, tag=tag + fx, name=tag + fx)
                x[tag] = t
                return t

            def drain_A():
                nc.scalar.activation(
                    mat("A")[:], x["a_ps"][:], mybir.ActivationFunctionType.Copy
                )

            def zy1():
                nc.vector.tensor_tensor(
                    mat("ZY1")[:], icons[:, 0, :], x["A"][:],
                    op=mybir.AluOpType.subtract,
                )

            def mk_prod(dst, u, v):
                def f():
                    x[dst] = prod(x[u], x[v])
                return f

            def drain(dst, src, eng):
                def f():
                    t = mat(dst)
                    if eng == "act":
                        nc.scalar.activation(
                            t[:], x[src][:], mybir.ActivationFunctionType.Copy
                        )
                    else:
                        nc.vector.tensor_copy(t[:], x[src][:])
                return f

            def sub(dst, k, src):
                def f():
                    nc.vector.tensor_tensor(
                        mat(dst)[:], icons[:, k, :], x[src][:],
                        op=mybir.AluOpType.subtract,
                    )
                return f

            def fstore():
                f_ps, abv = x["f_ps"], x["abv"]
                nc.vector.tensor_scalar_mul(
                    ftiles[0][:, b, :], f_ps[:, 0:C], abv[:, 1:2]
                )
                nc.scalar.activation(
                    ftiles[1][:, b, :], f_ps[:, C : 2 * C],
                    mybir.ActivationFunctionType.Copy, scale=abv[:, 1:2],
                )

            return [
                load, stats, trace, transpose, scale_xcT, gram,
                drain_A, zy1,
                mk_prod("y1_ps", "A", "ZY1"), drain("Y1", "y1_ps", "act"),
                mk_prod("w1_ps", "ZY1", "Y1"), sub("ZY2", 1, "w1_ps"),
                mk_prod("y2_ps", "Y1", "ZY2"), drain("Y2", "y2_ps", "dve"),
                mk_prod("z2_ps", "ZY2", "ZY1"), drain("Z2", "z2_ps", "act"),
                mk_prod("w2_ps", "Z2", "Y2"), sub("ZY3", 2, "w2_ps"),
                mk_prod("f_ps", "Y2", "ZY3"), fstore,
            ]

        for b0 in range(0, n_samples, 3):
            grp = [sample_stages(b) for b in range(b0, min(b0 + 3, n_samples))]
            n = len(grp[0])
            for step in range(n + 2):
                for i, sg in enumerate(grp):
                    if 0 <= step - i < n:
                        sg[step - i]()

        # ---- flush: one DMA per matrix row, all samples at once ----
        for r in range(C):
            L = C - r
            s0 = int(rowstart[r])
            src = ftiles[r // P][r % P : r % P + 1, :, r:C]
            # measured issue rates: gpsimd 0.59us, sync 0.77us, scalar 0.83us
            m = r % 10
            if m in (0, 2, 4, 6):
                eng = nc.gpsimd
            elif m in (1, 5, 8):
                eng = nc.sync
            else:
                eng = nc.scalar
            eng.dma_start(y_ap[:, s0 : s0 + L], src)


def _make_const_inputs():
    # icons[:, k, :]: [3I, 12I, 768I] in concatenated row-tile layout:
    # cols 0:256 = matrix rows 0:128 (diag at col p),
    # cols 256:512 = matrix rows 128:256 (diag at col 256+128+p).
    e = np.zeros((P, 2 * C), np.float32)
    e[np.arange(P), np.arange(P)] = 1.0
    e[np.arange(P), C + P + np.arange(P)] = 1.0
    icons = np.stack([3.0 * e, 12.0 * e, 768.0 * e], axis=1).astype(np.float16)
    return {
        "ident": np.eye(P, dtype=np.float16),
        "icons": np.ascontiguousarray(icons),
        "ones": np.ones((P, 1), np.float32),
        "onesrow": np.ones((1, P), np.float32),
    }


def make_nc(n_samples=S, num_devices=NCORES):
    nc = bacc.Bacc(
        "TRN2",
        target_bir_lowering=False,
        debug=False,
        enable_asserts=False,
        num_devices=num_devices,
    )
    x_ap = nc.dram_tensor("x", (n_samples, C, M), F32, kind="ExternalInput").ap()
    y_ap = nc.dram_tensor("y", (n_samples, NTRIU), F32, kind="ExternalOutput").ap()
    ident_ap = nc.dram_tensor("ident", (P, P), MM_DT, kind="ExternalInput").ap()
    icons_ap = nc.dram_tensor("icons", (P, 3, 2 * C), MM_DT, kind="ExternalInput").ap()
    ones_ap = nc.dram_tensor("ones", (P, 1), F32, kind="ExternalInput").ap()
    onesrow_ap = nc.dram_tensor("onesrow", (1, P), F32, kind="ExternalInput").ap()
    with tile.TileContext(nc) as tc:
        build(tc, y_ap, x_ap, ident_ap, icons_ap, ones_ap, onesrow_ap, n_samples)
    nc.compile()
    return nc


def kernel(x, _trace=False, **_trace_kwargs):
    global LAST_EXEC_NS, LAST_RESULTS
    x = np.ascontiguousarray(np.asarray(x), dtype=np.float32)
    assert x.shape == (B, C, 14, 14)
    xr = x.reshape(B, C, M)

    nc = make_nc()
    consts = _make_const_inputs()
    in_maps = [
        {"x": np.ascontiguousarray(xr[i * S : (i + 1) * S]), **consts}
        for i in range(NCORES)
    ]
    res = bass_utils.run_bass_kernel_spmd(
        nc, in_maps, core_ids=list(range(NCORES)), trace=_trace, **_trace_kwargs
    )
    LAST_EXEC_NS = res.exec_time_ns
    LAST_RESULTS = res
    return np.concatenate([r["y"] for r in res.results], axis=0)

